# revision 48
# baseline (speedup 1.0000x reference)
"""Spatial-reduction attention (PVT-style) on 8 Trainium2 NeuronCores.

Shapes (hardcoded): x [4, 4096, 512], 8 heads, head_dim 64, SR=2 conv
reduction -> 1024 keys. Sharding: core c handles batch c//2, query half
c%2 (2048 queries). Conv + kv are recomputed per core pair.

Per-core dataflow (everything kept transposed, [channel, token]):
  qT   = q_wT.T @ xq            [512, 2048]   (fp8 DoubleRow)
  convT= sum_ij srw_ij.T @ gather_ij(xf) + sr_b   [512, 1024] (fp8 DR)
  kT   = k_wT.T @ convT8        [512, 1024]   (fp8 DR)
  v    = convT.T @ v_wT         [1024, 512]   (bf16; stored fp8 +ones col)
  ST_h = kT_h.T @ qT_h          [1024, 2048] per head (bf16)
  E    = exp(ST * scale)        (ScalarE f8 out; some key-tiles on DVE
                                 via Schraudolph int16/bf16 bitcast)
  O_h  = v_aug_h.T @ E          [65, 2048] fp8 DoubleRow over key pairs
  OT   = O_h / denom            [512, 2048] bf16 (recip DVE, bcast+mul GpSimd)
  PT   = proj_wT.T @ OT + proj_b  [512, 2048] bf16 -> fp32 out, interleaved
"""

import numpy as np
import ml_dtypes
from contextlib import ExitStack

import concourse.bass as bass
import concourse.mybir as mybir
from concourse import bacc
from concourse.bass_utils import run_bass_kernel_spmd
from concourse.tile import TileContext

BF = mybir.dt.bfloat16
F8 = mybir.dt.float8e4
F32 = mybir.dt.float32
I16 = mybir.dt.int16
P = 128
CT = 4            # channel tiles (512 / 128)
NQ = 2048         # queries per core
NKT = 8           # key tiles (1024 / 128)
SCALE = 0.125     # 64 ** -0.5

# ---- precision / engine-assignment config ----
Q_FP8 = False     # phase B (q proj) fp8 DoubleRow
CONV_FP8 = False  # phase C (conv) fp8 DoubleRow
K_FP8 = False     # phase D (k proj) fp8 DoubleRow
O_FP8 = True      # phase F attn@V fp8 DoubleRow (e + v in fp8)
V_SPLIT = True    # represent v as f8 hi + f8 residual (two DR accumulations)
SCHR_NK = (1, 4, 6)      # key tiles whose exp runs on DVE (Schraudolph)
SCHR_A = SCALE * 128.0 / float(np.log(2.0))
SCHR_B = 16256.0 - 7.5
# int8 variant: bitcast lands directly in fp8e4 (1 DVE op, no cast)
SCHR_A8 = SCALE * 8.0 / float(np.log(2.0))
SCHR_B8 = 55.65

DT_Q = F8 if Q_FP8 else BF
DT_C = F8 if CONV_FP8 else BF
DT_K = F8 if K_FP8 else BF
DT_E = F8 if O_FP8 else BF
DR = mybir.MatmulPerfMode.DoubleRow
Exp = mybir.ActivationFunctionType.Exp
MULT = mybir.AluOpType.mult
ADD = mybir.AluOpType.add

_CACHE = {}


def _build_program():
    nc = bacc.Bacc("TRN2", target_bir_lowering=False, debug=False, num_devices=8)

    xq_d = nc.dram_tensor("xq", [512, NQ], DT_Q, kind="ExternalInput")
    xf_d = nc.dram_tensor("xf", [512, 4096], DT_C, kind="ExternalInput")
    qw_d = nc.dram_tensor("qw", [512, 512], DT_Q, kind="ExternalInput")     # [c, dq]
    kw_d = nc.dram_tensor("kw", [512, 512], DT_K, kind="ExternalInput")     # [c, dk]
    vw_d = nc.dram_tensor("vw", [512, 512], BF, kind="ExternalInput")       # [c, dv]
    srw_d = nc.dram_tensor("srw", [4, 512, 512], DT_C, kind="ExternalInput")  # [ij, ci, co]
    srb_d = nc.dram_tensor("srb", [512], F32, kind="ExternalInput")
    pw_d = nc.dram_tensor("pw", [512, 512], BF, kind="ExternalInput")       # [c, co]
    pb_d = nc.dram_tensor("pb", [512], F32, kind="ExternalInput")
    out_d = nc.dram_tensor("out_t", [512, NQ], F32, kind="ExternalOutput")

    with TileContext(nc) as tc, ExitStack() as ctx:
        const = ctx.enter_context(tc.tile_pool(name="const", bufs=1))
        ep = ctx.enter_context(tc.tile_pool(name="ep", bufs=2))
        e16p = ctx.enter_context(tc.tile_pool(name="e16p", bufs=3))
        rrp = ctx.enter_context(tc.tile_pool(name="rrp", bufs=1))
        rbp = ctx.enter_context(tc.tile_pool(name="rbp", bufs=1))
        outp = ctx.enter_context(tc.tile_pool(name="outp", bufs=2))

        dma = nc.sync.dma_start

        # ---- load inputs (order matters: B needs qw+xq, C needs srw+xf) ----
        qw_sb = const.tile([P, CT, 512], DT_Q)
        qw_r = qw_d.rearrange("(t p) n -> p t n", p=P)
        for t in range(CT):
            dma(out=qw_sb[:, t, :], in_=qw_r[:, t, :])
        xq_sb = const.tile([P, CT, NQ], DT_Q)
        xq_r = xq_d.rearrange("(t p) n -> p t n", p=P)
        for h in range(2):
            for t in range(CT):
                dma(out=xq_sb[:, t, h * 1024:(h + 1) * 1024],
                    in_=xq_r[:, t, h * 1024:(h + 1) * 1024])
        srw_sb = const.tile([P, 4, CT, 512], DT_C)
        srw_r = srw_d.rearrange("i (t p) o -> p i t o", p=P)
        for ij4 in range(4):
            dma(out=srw_sb[:, ij4, :, :], in_=srw_r[:, ij4, :, :])
        xf_sb = const.tile([P, CT, 4096], DT_C)
        xf_r = xf_d.rearrange("(t p) n -> p t n", p=P)
        for t in range(CT):
            dma(out=xf_sb[:, t, :], in_=xf_r[:, t, :])
        kw_sb = const.tile([P, CT, 512], DT_K)
        dma(out=kw_sb, in_=kw_d.rearrange("(t p) n -> p t n", p=P))
        vw_sb = const.tile([P, CT, 512], BF)
        dma(out=vw_sb, in_=vw_d.rearrange("(t p) n -> p t n", p=P))
        pw_sb = const.tile([P, CT, 512], BF)
        dma(out=pw_sb, in_=pw_d.rearrange("(t p) n -> p t n", p=P))
        srb_sb = const.tile([P, CT], F32)
        dma(out=srb_sb, in_=srb_d.rearrange("(t p) -> p t", p=P))
        pb_sb = const.tile([P, CT], F32)
        dma(out=pb_sb, in_=pb_d.rearrange("(t p) -> p t", p=P))

        qT_sb = const.tile([P, CT, NQ], BF)
        convT_sb = const.tile([P, CT, 1024], BF)
        if K_FP8:
            convT8_sb = const.tile([P, CT, 1024], F8)
        kTz_sb = const.tile([P, 8, 1024], BF)
        vaug_sb = const.tile([P, NKT, 8, 128], DT_E)
        if V_SPLIT:
            vlo_sb = const.tile([P, NKT, 8, 64], F8)
        oT_sb = const.tile([P, CT, NQ], BF)

        nc.gpsimd.memset(vaug_sb, 0.0)
        nc.gpsimd.memset(vaug_sb[:, :, :, 64:65], 1.0)
        nc.gpsimd.memset(kTz_sb, 0.0)

        with ExitStack() as ps_ctx:
            ps1 = ps_ctx.enter_context(tc.tile_pool(name="ps1", bufs=6, space="PSUM"))

            # ---- phase B: qT = q_wT.T @ xq ----
            for dq in range(CT):
                for nqb in range(4):
                    ps = ps1.tile([P, 512], F32)
                    if Q_FP8:
                        for cp in range(2):
                            nc.tensor.matmul(
                                ps,
                                qw_sb[:, 2 * cp:2 * cp + 2, dq * 128:(dq + 1) * 128],
                                xq_sb[:, 2 * cp:2 * cp + 2, nqb * 512:(nqb + 1) * 512],
                                start=(cp == 0), stop=(cp == 1),
                                perf_mode=DR,
                            )
                    else:
                        for c in range(CT):
                            nc.tensor.matmul(
                                ps,
                                qw_sb[:, c, dq * 128:(dq + 1) * 128],
                                xq_sb[:, c, nqb * 512:(nqb + 1) * 512],
                                start=(c == 0), stop=(c == CT - 1),
                            )
                    nc.scalar.copy(qT_sb[:, dq, nqb * 512:(nqb + 1) * 512], ps)

            # ---- phase C: convT (spatial reduction) ----
            xv = xf_sb[:, :, :].rearrange(
                "p c (a i b j) -> p c i j a b", a=32, i=2, b=32, j=2)
            for co in range(CT):
                for nkb in range(2):
                    ps = ps1.tile([P, 512], F32)
                    n_mm = 0
                    n_tot = 8 if CONV_FP8 else 16
                    for ij in range(4):
                        i, j = ij >> 1, ij & 1
                        if CONV_FP8:
                            for cp in range(2):
                                rhs = xv[:, 2 * cp:2 * cp + 2, i, j,
                                         nkb * 16:(nkb + 1) * 16, :]
                                nc.tensor.matmul(
                                    ps,
                                    srw_sb[:, ij, 2 * cp:2 * cp + 2,
                                           co * 128:(co + 1) * 128],
                                    rhs,
                                    start=(n_mm == 0), stop=(n_mm == n_tot - 1),
                                    perf_mode=DR,
                                )
                                n_mm += 1
                        else:
                            for ci in range(CT):
                                rhs = xv[:, ci, i, j, nkb * 16:(nkb + 1) * 16, :]
                                nc.tensor.matmul(
                                    ps,
                                    srw_sb[:, ij, ci, co * 128:(co + 1) * 128],
                                    rhs,
                                    start=(n_mm == 0), stop=(n_mm == n_tot - 1),
                                )
                                n_mm += 1
                    nc.vector.tensor_scalar_add(
                        convT_sb[:, co, nkb * 512:(nkb + 1) * 512],
                        ps, srb_sb[:, co:co + 1])
                    if K_FP8:
                        nc.vector.tensor_copy(
                            convT8_sb[:, co, nkb * 512:(nkb + 1) * 512],
                            convT_sb[:, co, nkb * 512:(nkb + 1) * 512])

            # ---- phase D: kT = k_wT.T @ convT ----
            for kt in range(CT):
                for nkb in range(2):
                    ps = ps1.tile([P, 512], F32)
                    if K_FP8:
                        for cp in range(2):
                            nc.tensor.matmul(
                                ps,
                                kw_sb[:, 2 * cp:2 * cp + 2, kt * 128:(kt + 1) * 128],
                                convT8_sb[:, 2 * cp:2 * cp + 2,
                                          nkb * 512:(nkb + 1) * 512],
                                start=(cp == 0), stop=(cp == 1),
                                perf_mode=DR,
                            )
                    else:
                        for c in range(CT):
                            nc.tensor.matmul(
                                ps,
                                kw_sb[:, c, kt * 128:(kt + 1) * 128],
                                convT_sb[:, c, nkb * 512:(nkb + 1) * 512],
                                start=(c == 0), stop=(c == CT - 1),
                            )
                    nc.scalar.copy(
                        kTz_sb[0:64, 2 * kt, nkb * 512:(nkb + 1) * 512],
                        ps[0:64, :])
                    nc.scalar.copy(
                        kTz_sb[64:128, 2 * kt + 1, nkb * 512:(nkb + 1) * 512],
                        ps[64:128, :])

            # ---- phase E: v = convT.T @ v_wT (natural layout + ones col) ----
            for nk in range(NKT):
                ps = ps1.tile([P, 512], F32)
                for c in range(CT):
                    nc.tensor.matmul(
                        ps,
                        convT_sb[:, c, nk * 128:(nk + 1) * 128],
                        vw_sb[:, c, :],
                        start=(c == 0), stop=(c == CT - 1),
                    )
                nc.vector.tensor_copy(
                    vaug_sb[:, nk, :, 0:64],
                    ps.rearrange("p (h e) -> p h e", e=64),
                )
                if V_SPLIT:
                    # f8 residual: vlo = v - round_f8(v)
                    nc.vector.tensor_sub(
                        vlo_sb[:, nk, :, :],
                        ps.rearrange("p (h e) -> p h e", e=64),
                        vaug_sb[:, nk, :, 0:64],
                    )

        # ---- phase F: attention, query-chunk x head-pair, proj interleaved ----
        with ExitStack() as ps_ctx:
            ps_s = ps_ctx.enter_context(
                tc.tile_pool(name="ps_s", bufs=2, space="PSUM"))
            ps_o = ps_ctx.enter_context(
                tc.tile_pool(name="ps_o", bufs=1, space="PSUM"))
            ps_g = ps_ctx.enter_context(
                tc.tile_pool(name="ps_g", bufs=2, space="PSUM"))

            def emit_G(qci, co):
                ps = ps_g.tile([P, 512], F32, tag="g", name=f"g_{qci}_{co}")
                for c in range(CT):
                    nc.tensor.matmul(
                        ps,
                        pw_sb[:, c, co * 128:(co + 1) * 128],
                        oT_sb[:, c, qci * 512:(qci + 1) * 512],
                        start=(c == 0), stop=(c == CT - 1),
                    )
                pt = outp.tile([P, 512], F32)
                nc.vector.tensor_scalar_add(pt, ps, pb_sb[:, co:co + 1])
                dma(out=out_d[co * 128:(co + 1) * 128,
                              qci * 512:(qci + 1) * 512], in_=pt)

            for qc in range(4):
                for pr in range(4):
                    # proj tile of the previous query chunk goes FIRST: it
                    # depends on long-finished data, so it keeps the PE warm
                    # while this iteration's exp/normalize chains start up.
                    if qc > 0:
                        emit_G(qc - 1, pr)
                    qmv = qT_sb[:, pr, qc * 512:(qc + 1) * 512]
                    e = ep.tile([P, NKT, 1024], DT_E, tag="e",
                                name=f"e_{qc}_{pr}")
                    oA = ps_o.tile([P, 512], F32, tag="oA", name=f"oA_{qc}_{pr}")
                    oB = ps_o.tile([P, 512], F32, tag="oB", name=f"oB_{qc}_{pr}")

                    def emit_O(nk, first, last):
                        # issued a couple of nk behind so PE never waits on exp
                        if O_FP8:
                            for h2, o_ps, cols in ((0, oA, slice(0, 512)),
                                                   (1, oB, slice(512, 1024))):
                                hi = lambda st, sp: nc.tensor.matmul(
                                    o_ps, vaug_sb[:, nk - 1:nk + 1, 2 * pr + h2, :],
                                    e[:, nk - 1:nk + 1, cols],
                                    start=st, stop=sp, perf_mode=DR)
                                lo = lambda: nc.tensor.matmul(
                                    o_ps[0:64, :],
                                    vlo_sb[:, nk - 1:nk + 1, 2 * pr + h2, :],
                                    e[:, nk - 1:nk + 1, cols],
                                    start=False, stop=False, perf_mode=DR,
                                    skip_group_check=True)
                                if not V_SPLIT:
                                    hi(first, last)
                                elif last:
                                    lo()
                                    hi(first, True)
                                else:
                                    hi(first, False)
                                    lo()
                        else:
                            nc.tensor.matmul(
                                oA, vaug_sb[:, nk, 2 * pr, :],
                                e[:, nk, 0:512], start=first, stop=last)
                            nc.tensor.matmul(
                                oB, vaug_sb[:, nk, 2 * pr + 1, :],
                                e[:, nk, 512:1024], start=first, stop=last)

                    pending = []
                    emitted = [0]
                    o_units = [nk for nk in range(NKT)
                               if (not O_FP8) or nk % 2 == 1]

                    def flush(keep):
                        while len(pending) > keep:
                            p_nk = pending.pop(0)
                            emit_O(p_nk, emitted[0] == 0,
                                   emitted[0] == len(o_units) - 1)
                            emitted[0] += 1

                    for nk in range(NKT):
                        s = ps_s.tile([P, 1024], F32, tag="s",
                                      name=f"s_{qc}_{pr}_{nk}")
                        nc.tensor.matmul(
                            s[:, 0:512],
                            kTz_sb[:, 2 * pr, nk * 128:(nk + 1) * 128],
                            qmv, start=True, stop=True)
                        nc.tensor.matmul(
                            s[:, 512:1024],
                            kTz_sb[:, 2 * pr + 1, nk * 128:(nk + 1) * 128],
                            qmv, start=True, stop=True)
                        flush(1)
                        if nk in SCHR_NK:
                            if DT_E == BF:
                                nc.vector.tensor_scalar(
                                    e[:, nk, :].bitcast(I16), s,
                                    SCHR_A, SCHR_B, MULT, ADD)
                            else:
                                nc.vector.tensor_scalar(
                                    e[:, nk, :].bitcast(mybir.dt.int8), s,
                                    SCHR_A8, SCHR_B8, MULT, ADD)
                        else:
                            nc.scalar.activation(e[:, nk, :], s, Exp, scale=SCALE)
                        if (not O_FP8) or nk % 2 == 1:
                            pending.append(nk)
                    flush(0)
                    # normalize: OT = O / denom (denom = row 64 of oA/oB)
                    dn = rrp.tile([1, 1024], F32, tag="dn", name=f"dn_{qc}_{pr}")
                    nc.vector.tensor_copy(dn[0:1, 0:512], oA[64:65, :])
                    nc.vector.tensor_copy(dn[0:1, 512:1024], oB[64:65, :])
                    rr = rrp.tile([1, 1024], F32, tag="rr", name=f"rr_{qc}_{pr}")
                    nc.vector.reciprocal_approx_fast(out=rr, in_=dn)
                    rb = rbp.tile([64, 1024], F32, tag="rb", name=f"rb_{qc}_{pr}")
                    nc.gpsimd.partition_broadcast(rb[:, 0:512], rr[0:1, 0:512])
                    nc.gpsimd.partition_broadcast(rb[:, 512:1024], rr[0:1, 512:1024])
                    nc.vector.tensor_mul(
                        oT_sb[0:64, pr, qc * 512:(qc + 1) * 512],
                        oA[0:64, :], rb[:, 0:512])
                    nc.vector.tensor_mul(
                        oT_sb[64:128, pr, qc * 512:(qc + 1) * 512],
                        oB[0:64, :], rb[:, 512:1024])
            for co in range(CT):
                emit_G(3, co)

    nc.compile()
    return nc


def kernel(x, q_w, kv_w, sr_w, sr_b, proj_w, proj_b, H=64, W=64, **_kw):
    x = np.asarray(x, dtype=np.float32)
    q_w = np.asarray(q_w, dtype=np.float32)
    kv_w = np.asarray(kv_w, dtype=np.float32)
    sr_w = np.asarray(sr_w, dtype=np.float32)
    sr_b = np.asarray(sr_b, dtype=np.float32)
    proj_w = np.asarray(proj_w, dtype=np.float32)
    proj_b = np.asarray(proj_b, dtype=np.float32)
    B, N, C = x.shape

    if "nc" not in _CACHE:
        _CACHE["nc"] = _build_program()
    nc = _CACHE["nc"]

    bf = ml_dtypes.bfloat16
    f8 = ml_dtypes.float8_e4m3
    np_q = f8 if Q_FP8 else bf
    np_c = f8 if CONV_FP8 else bf
    np_k = f8 if K_FP8 else bf

    qw_t = np.ascontiguousarray(q_w.T).astype(np_q)              # [c, dq]
    kw_t = np.ascontiguousarray(kv_w[:512].T).astype(np_k)       # [c, dk]
    vw_t = np.ascontiguousarray(kv_w[512:].T).astype(bf)         # [c, dv]
    srw_t = np.ascontiguousarray(
        sr_w.transpose(2, 3, 1, 0).reshape(4, 512, 512)).astype(np_c)
    pw_t = np.ascontiguousarray(proj_w.T).astype(bf)             # [c, co]

    in_maps = []
    xT = np.ascontiguousarray(x.transpose(0, 2, 1))              # [B, C, N] f32
    for c in range(8):
        b, hf = c // 2, c % 2
        in_maps.append({
            "xq": np.ascontiguousarray(
                xT[b][:, hf * NQ:(hf + 1) * NQ]).astype(np_q),
            "xf": xT[b].astype(np_c),
            "qw": qw_t, "kw": kw_t, "vw": vw_t,
            "srw": srw_t, "srb": sr_b,
            "pw": pw_t, "pb": proj_b,
        })

    res = run_bass_kernel_spmd(nc, in_maps, core_ids=list(range(8)))
    _CACHE["last_exec_time_ns"] = res.exec_time_ns

    out = np.empty((B, N, C), dtype=np.float32)
    for c in range(8):
        b, hf = c // 2, c % 2
        out[b, hf * NQ:(hf + 1) * NQ, :] = res.results[c]["out_t"].T
    return out


# revision 49
# speedup vs baseline: 1.0041x; 1.0041x over previous
"""Spatial-reduction attention (PVT-style) on 8 Trainium2 NeuronCores.

Shapes (hardcoded): x [4, 4096, 512], 8 heads, head_dim 64, SR=2 conv
reduction -> 1024 keys. Sharding: core c handles batch c//2, query half
c%2 (2048 queries). Conv + kv are recomputed per core pair.

Per-core dataflow (everything kept transposed, [channel, token]):
  qT   = q_wT.T @ xq            [512, 2048]   (fp8 DoubleRow)
  convT= sum_ij srw_ij.T @ gather_ij(xf) + sr_b   [512, 1024] (fp8 DR)
  kT   = k_wT.T @ convT8        [512, 1024]   (fp8 DR)
  v    = convT.T @ v_wT         [1024, 512]   (bf16; stored fp8 +ones col)
  ST_h = kT_h.T @ qT_h          [1024, 2048] per head (bf16)
  E    = exp(ST * scale)        (ScalarE f8 out; some key-tiles on DVE
                                 via Schraudolph int16/bf16 bitcast)
  O_h  = v_aug_h.T @ E          [65, 2048] fp8 DoubleRow over key pairs
  OT   = O_h / denom            [512, 2048] bf16 (recip DVE, bcast+mul GpSimd)
  PT   = proj_wT.T @ OT + proj_b  [512, 2048] bf16 -> fp32 out, interleaved
"""

import numpy as np
import ml_dtypes
from contextlib import ExitStack

import concourse.bass as bass
import concourse.mybir as mybir
from concourse import bacc
from concourse.bass_utils import run_bass_kernel_spmd
from concourse.tile import TileContext

BF = mybir.dt.bfloat16
F8 = mybir.dt.float8e4
F32 = mybir.dt.float32
I16 = mybir.dt.int16
P = 128
CT = 4            # channel tiles (512 / 128)
NQ = 2048         # queries per core
NKT = 8           # key tiles (1024 / 128)
SCALE = 0.125     # 64 ** -0.5

# ---- precision / engine-assignment config ----
Q_FP8 = False     # phase B (q proj) fp8 DoubleRow
CONV_FP8 = False  # phase C (conv) fp8 DoubleRow
K_FP8 = False     # phase D (k proj) fp8 DoubleRow
O_FP8 = True      # phase F attn@V fp8 DoubleRow (e + v in fp8)
V_SPLIT = True    # represent v as f8 hi + f8 residual (two DR accumulations)
SCHR_NK = (1, 4, 6)      # key tiles whose exp runs on DVE (Schraudolph)
SCHR_A = SCALE * 128.0 / float(np.log(2.0))
SCHR_B = 16256.0 - 7.5
# int8 variant: bitcast lands directly in fp8e4 (1 DVE op, no cast)
SCHR_A8 = SCALE * 8.0 / float(np.log(2.0))
SCHR_B8 = 55.65

DT_Q = F8 if Q_FP8 else BF
DT_C = F8 if CONV_FP8 else BF
DT_K = F8 if K_FP8 else BF
DT_E = F8 if O_FP8 else BF
DR = mybir.MatmulPerfMode.DoubleRow
Exp = mybir.ActivationFunctionType.Exp
MULT = mybir.AluOpType.mult
ADD = mybir.AluOpType.add

_CACHE = {}


def _build_program():
    nc = bacc.Bacc("TRN2", target_bir_lowering=False, debug=False, num_devices=8)

    xq_d = nc.dram_tensor("xq", [512, NQ], DT_Q, kind="ExternalInput")
    xf_d = nc.dram_tensor("xf", [512, 4096], DT_C, kind="ExternalInput")
    qw_d = nc.dram_tensor("qw", [512, 512], DT_Q, kind="ExternalInput")     # [c, dq]
    kw_d = nc.dram_tensor("kw", [512, 512], DT_K, kind="ExternalInput")     # [c, dk]
    vw_d = nc.dram_tensor("vw", [512, 512], BF, kind="ExternalInput")       # [c, dv]
    srw_d = nc.dram_tensor("srw", [4, 512, 512], DT_C, kind="ExternalInput")  # [ij, ci, co]
    srb_d = nc.dram_tensor("srb", [512], F32, kind="ExternalInput")
    pw_d = nc.dram_tensor("pw", [512, 512], BF, kind="ExternalInput")       # [c, co]
    pb_d = nc.dram_tensor("pb", [512], F32, kind="ExternalInput")
    out_d = nc.dram_tensor("out_t", [512, NQ], F32, kind="ExternalOutput")

    with TileContext(nc) as tc, ExitStack() as ctx:
        const = ctx.enter_context(tc.tile_pool(name="const", bufs=1))
        ep = ctx.enter_context(tc.tile_pool(name="ep", bufs=2))
        rrp = ctx.enter_context(tc.tile_pool(name="rrp", bufs=2))
        rbp = ctx.enter_context(tc.tile_pool(name="rbp", bufs=2))
        outp = ctx.enter_context(tc.tile_pool(name="outp", bufs=3))

        dma = nc.sync.dma_start

        # ---- load inputs (order matters: B needs qw+xq, C needs srw+xf) ----
        qw_sb = const.tile([P, CT, 512], DT_Q)
        qw_r = qw_d.rearrange("(t p) n -> p t n", p=P)
        for t in range(CT):
            dma(out=qw_sb[:, t, :], in_=qw_r[:, t, :])
        xq_sb = const.tile([P, CT, NQ], DT_Q)
        xq_r = xq_d.rearrange("(t p) n -> p t n", p=P)
        for h in range(2):
            for t in range(CT):
                dma(out=xq_sb[:, t, h * 1024:(h + 1) * 1024],
                    in_=xq_r[:, t, h * 1024:(h + 1) * 1024])
        srw_sb = const.tile([P, 4, CT, 512], DT_C)
        srw_r = srw_d.rearrange("i (t p) o -> p i t o", p=P)
        for ij4 in range(4):
            dma(out=srw_sb[:, ij4, :, :], in_=srw_r[:, ij4, :, :])
        xf_sb = const.tile([P, CT, 4096], DT_C)
        xf_r = xf_d.rearrange("(t p) n -> p t n", p=P)
        for t in range(CT):
            dma(out=xf_sb[:, t, :], in_=xf_r[:, t, :])
        kw_sb = const.tile([P, CT, 512], DT_K)
        dma(out=kw_sb, in_=kw_d.rearrange("(t p) n -> p t n", p=P))
        vw_sb = const.tile([P, CT, 512], BF)
        dma(out=vw_sb, in_=vw_d.rearrange("(t p) n -> p t n", p=P))
        pw_sb = const.tile([P, CT, 512], BF)
        dma(out=pw_sb, in_=pw_d.rearrange("(t p) n -> p t n", p=P))
        srb_sb = const.tile([P, CT], F32)
        dma(out=srb_sb, in_=srb_d.rearrange("(t p) -> p t", p=P))
        pb_sb = const.tile([P, CT], F32)
        dma(out=pb_sb, in_=pb_d.rearrange("(t p) -> p t", p=P))

        qT_sb = const.tile([P, CT, NQ], BF)
        convT_sb = const.tile([P, CT, 1024], BF)
        if K_FP8:
            convT8_sb = const.tile([P, CT, 1024], F8)
        kTz_sb = const.tile([P, 8, 1024], BF)
        vaug_sb = const.tile([P, NKT, 8, 128], DT_E)
        if V_SPLIT:
            vlo_sb = const.tile([P, NKT, 8, 64], F8)
        oT_sb = const.tile([P, CT, NQ], BF)

        nc.gpsimd.memset(vaug_sb, 0.0)
        nc.gpsimd.memset(vaug_sb[:, :, :, 64:65], 1.0)
        nc.gpsimd.memset(kTz_sb, 0.0)

        with ExitStack() as ps_ctx:
            ps1 = ps_ctx.enter_context(tc.tile_pool(name="ps1", bufs=6, space="PSUM"))

            # ---- phase B: qT = q_wT.T @ xq ----
            for dq in range(CT):
                for nqb in range(4):
                    ps = ps1.tile([P, 512], F32)
                    if Q_FP8:
                        for cp in range(2):
                            nc.tensor.matmul(
                                ps,
                                qw_sb[:, 2 * cp:2 * cp + 2, dq * 128:(dq + 1) * 128],
                                xq_sb[:, 2 * cp:2 * cp + 2, nqb * 512:(nqb + 1) * 512],
                                start=(cp == 0), stop=(cp == 1),
                                perf_mode=DR,
                            )
                    else:
                        for c in range(CT):
                            nc.tensor.matmul(
                                ps,
                                qw_sb[:, c, dq * 128:(dq + 1) * 128],
                                xq_sb[:, c, nqb * 512:(nqb + 1) * 512],
                                start=(c == 0), stop=(c == CT - 1),
                            )
                    nc.scalar.copy(qT_sb[:, dq, nqb * 512:(nqb + 1) * 512], ps)

            # ---- phase C: convT (spatial reduction) ----
            xv = xf_sb[:, :, :].rearrange(
                "p c (a i b j) -> p c i j a b", a=32, i=2, b=32, j=2)
            for co in range(CT):
                for nkb in range(2):
                    ps = ps1.tile([P, 512], F32)
                    n_mm = 0
                    n_tot = 8 if CONV_FP8 else 16
                    for ij in range(4):
                        i, j = ij >> 1, ij & 1
                        if CONV_FP8:
                            for cp in range(2):
                                rhs = xv[:, 2 * cp:2 * cp + 2, i, j,
                                         nkb * 16:(nkb + 1) * 16, :]
                                nc.tensor.matmul(
                                    ps,
                                    srw_sb[:, ij, 2 * cp:2 * cp + 2,
                                           co * 128:(co + 1) * 128],
                                    rhs,
                                    start=(n_mm == 0), stop=(n_mm == n_tot - 1),
                                    perf_mode=DR,
                                )
                                n_mm += 1
                        else:
                            for ci in range(CT):
                                rhs = xv[:, ci, i, j, nkb * 16:(nkb + 1) * 16, :]
                                nc.tensor.matmul(
                                    ps,
                                    srw_sb[:, ij, ci, co * 128:(co + 1) * 128],
                                    rhs,
                                    start=(n_mm == 0), stop=(n_mm == n_tot - 1),
                                )
                                n_mm += 1
                    nc.vector.tensor_scalar_add(
                        convT_sb[:, co, nkb * 512:(nkb + 1) * 512],
                        ps, srb_sb[:, co:co + 1])
                    if K_FP8:
                        nc.vector.tensor_copy(
                            convT8_sb[:, co, nkb * 512:(nkb + 1) * 512],
                            convT_sb[:, co, nkb * 512:(nkb + 1) * 512])

            # ---- phase D: kT = k_wT.T @ convT ----
            for kt in range(CT):
                for nkb in range(2):
                    ps = ps1.tile([P, 512], F32)
                    if K_FP8:
                        for cp in range(2):
                            nc.tensor.matmul(
                                ps,
                                kw_sb[:, 2 * cp:2 * cp + 2, kt * 128:(kt + 1) * 128],
                                convT8_sb[:, 2 * cp:2 * cp + 2,
                                          nkb * 512:(nkb + 1) * 512],
                                start=(cp == 0), stop=(cp == 1),
                                perf_mode=DR,
                            )
                    else:
                        for c in range(CT):
                            nc.tensor.matmul(
                                ps,
                                kw_sb[:, c, kt * 128:(kt + 1) * 128],
                                convT_sb[:, c, nkb * 512:(nkb + 1) * 512],
                                start=(c == 0), stop=(c == CT - 1),
                            )
                    nc.scalar.copy(
                        kTz_sb[0:64, 2 * kt, nkb * 512:(nkb + 1) * 512],
                        ps[0:64, :])
                    nc.scalar.copy(
                        kTz_sb[64:128, 2 * kt + 1, nkb * 512:(nkb + 1) * 512],
                        ps[64:128, :])

            # ---- phase E: v = convT.T @ v_wT (natural layout + ones col) ----
            for nk in range(NKT):
                ps = ps1.tile([P, 512], F32)
                for c in range(CT):
                    nc.tensor.matmul(
                        ps,
                        convT_sb[:, c, nk * 128:(nk + 1) * 128],
                        vw_sb[:, c, :],
                        start=(c == 0), stop=(c == CT - 1),
                    )
                nc.vector.tensor_copy(
                    vaug_sb[:, nk, :, 0:64],
                    ps.rearrange("p (h e) -> p h e", e=64),
                )
                if V_SPLIT:
                    # f8 residual: vlo = v - round_f8(v)
                    nc.vector.tensor_sub(
                        vlo_sb[:, nk, :, :],
                        ps.rearrange("p (h e) -> p h e", e=64),
                        vaug_sb[:, nk, :, 0:64],
                    )

        # ---- phase F: attention, query-chunk x head-pair, proj interleaved ----
        with ExitStack() as ps_ctx:
            ps_s = ps_ctx.enter_context(
                tc.tile_pool(name="ps_s", bufs=2, space="PSUM"))
            ps_o = ps_ctx.enter_context(
                tc.tile_pool(name="ps_o", bufs=1, space="PSUM"))
            ps_g = ps_ctx.enter_context(
                tc.tile_pool(name="ps_g", bufs=2, space="PSUM"))

            def emit_G(qci, co):
                ps = ps_g.tile([P, 512], F32, tag="g", name=f"g_{qci}_{co}")
                for c in range(CT):
                    nc.tensor.matmul(
                        ps,
                        pw_sb[:, c, co * 128:(co + 1) * 128],
                        oT_sb[:, c, qci * 512:(qci + 1) * 512],
                        start=(c == 0), stop=(c == CT - 1),
                    )
                pt = outp.tile([P, 512], F32)
                nc.vector.tensor_scalar_add(pt, ps, pb_sb[:, co:co + 1])
                dma(out=out_d[co * 128:(co + 1) * 128,
                              qci * 512:(qci + 1) * 512], in_=pt)

            for qc in range(4):
                for pr in range(4):
                    # proj tile of the previous query chunk goes FIRST: it
                    # depends on long-finished data, so it keeps the PE warm
                    # while this iteration's exp/normalize chains start up.
                    if qc > 0:
                        emit_G(qc - 1, pr)
                    qmv = qT_sb[:, pr, qc * 512:(qc + 1) * 512]
                    e = ep.tile([P, NKT, 1024], DT_E, tag="e",
                                name=f"e_{qc}_{pr}")
                    oA = ps_o.tile([P, 512], F32, tag="oA", name=f"oA_{qc}_{pr}")
                    oB = ps_o.tile([P, 512], F32, tag="oB", name=f"oB_{qc}_{pr}")

                    def emit_O(nk, first, last):
                        # issued a couple of nk behind so PE never waits on exp
                        if O_FP8:
                            for h2, o_ps, cols in ((0, oA, slice(0, 512)),
                                                   (1, oB, slice(512, 1024))):
                                hi = lambda st, sp: nc.tensor.matmul(
                                    o_ps, vaug_sb[:, nk - 1:nk + 1, 2 * pr + h2, :],
                                    e[:, nk - 1:nk + 1, cols],
                                    start=st, stop=sp, perf_mode=DR)
                                lo = lambda: nc.tensor.matmul(
                                    o_ps[0:64, :],
                                    vlo_sb[:, nk - 1:nk + 1, 2 * pr + h2, :],
                                    e[:, nk - 1:nk + 1, cols],
                                    start=False, stop=False, perf_mode=DR,
                                    skip_group_check=True)
                                if not V_SPLIT:
                                    hi(first, last)
                                elif last:
                                    lo()
                                    hi(first, True)
                                else:
                                    hi(first, False)
                                    lo()
                        else:
                            nc.tensor.matmul(
                                oA, vaug_sb[:, nk, 2 * pr, :],
                                e[:, nk, 0:512], start=first, stop=last)
                            nc.tensor.matmul(
                                oB, vaug_sb[:, nk, 2 * pr + 1, :],
                                e[:, nk, 512:1024], start=first, stop=last)

                    pending = []
                    emitted = [0]
                    o_units = [nk for nk in range(NKT)
                               if (not O_FP8) or nk % 2 == 1]

                    def flush(keep):
                        while len(pending) > keep:
                            p_nk = pending.pop(0)
                            emit_O(p_nk, emitted[0] == 0,
                                   emitted[0] == len(o_units) - 1)
                            emitted[0] += 1

                    for nk in range(NKT):
                        s = ps_s.tile([P, 1024], F32, tag="s",
                                      name=f"s_{qc}_{pr}_{nk}")
                        nc.tensor.matmul(
                            s[:, 0:512],
                            kTz_sb[:, 2 * pr, nk * 128:(nk + 1) * 128],
                            qmv, start=True, stop=True)
                        nc.tensor.matmul(
                            s[:, 512:1024],
                            kTz_sb[:, 2 * pr + 1, nk * 128:(nk + 1) * 128],
                            qmv, start=True, stop=True)
                        flush(1)
                        if nk in SCHR_NK:
                            if DT_E == BF:
                                nc.vector.tensor_scalar(
                                    e[:, nk, :].bitcast(I16), s,
                                    SCHR_A, SCHR_B, MULT, ADD)
                            else:
                                nc.vector.tensor_scalar(
                                    e[:, nk, :].bitcast(mybir.dt.int8), s,
                                    SCHR_A8, SCHR_B8, MULT, ADD)
                        else:
                            nc.scalar.activation(e[:, nk, :], s, Exp, scale=SCALE)
                        if (not O_FP8) or nk % 2 == 1:
                            pending.append(nk)
                    flush(0)
                    # normalize: OT = O / denom (denom = row 64 of oA/oB)
                    dn = rrp.tile([1, 1024], F32, tag="dn", name=f"dn_{qc}_{pr}")
                    nc.vector.tensor_copy(dn[0:1, 0:512], oA[64:65, :])
                    nc.vector.tensor_copy(dn[0:1, 512:1024], oB[64:65, :])
                    rr = rrp.tile([1, 1024], F32, tag="rr", name=f"rr_{qc}_{pr}")
                    nc.vector.reciprocal_approx_fast(out=rr, in_=dn)
                    rb = rbp.tile([64, 1024], F32, tag="rb", name=f"rb_{qc}_{pr}")
                    nc.gpsimd.partition_broadcast(rb[:, 0:512], rr[0:1, 0:512])
                    nc.gpsimd.partition_broadcast(rb[:, 512:1024], rr[0:1, 512:1024])
                    nc.vector.tensor_mul(
                        oT_sb[0:64, pr, qc * 512:(qc + 1) * 512],
                        oA[0:64, :], rb[:, 0:512])
                    nc.vector.tensor_mul(
                        oT_sb[64:128, pr, qc * 512:(qc + 1) * 512],
                        oB[0:64, :], rb[:, 512:1024])
            for co in range(CT):
                emit_G(3, co)

    nc.compile()
    return nc


def kernel(x, q_w, kv_w, sr_w, sr_b, proj_w, proj_b, H=64, W=64, **_kw):
    x = np.asarray(x, dtype=np.float32)
    q_w = np.asarray(q_w, dtype=np.float32)
    kv_w = np.asarray(kv_w, dtype=np.float32)
    sr_w = np.asarray(sr_w, dtype=np.float32)
    sr_b = np.asarray(sr_b, dtype=np.float32)
    proj_w = np.asarray(proj_w, dtype=np.float32)
    proj_b = np.asarray(proj_b, dtype=np.float32)
    B, N, C = x.shape

    if "nc" not in _CACHE:
        _CACHE["nc"] = _build_program()
    nc = _CACHE["nc"]

    bf = ml_dtypes.bfloat16
    f8 = ml_dtypes.float8_e4m3
    np_q = f8 if Q_FP8 else bf
    np_c = f8 if CONV_FP8 else bf
    np_k = f8 if K_FP8 else bf

    qw_t = np.ascontiguousarray(q_w.T).astype(np_q)              # [c, dq]
    kw_t = np.ascontiguousarray(kv_w[:512].T).astype(np_k)       # [c, dk]
    vw_t = np.ascontiguousarray(kv_w[512:].T).astype(bf)         # [c, dv]
    srw_t = np.ascontiguousarray(
        sr_w.transpose(2, 3, 1, 0).reshape(4, 512, 512)).astype(np_c)
    pw_t = np.ascontiguousarray(proj_w.T).astype(bf)             # [c, co]

    in_maps = []
    xT = np.ascontiguousarray(x.transpose(0, 2, 1))              # [B, C, N] f32
    for c in range(8):
        b, hf = c // 2, c % 2
        in_maps.append({
            "xq": np.ascontiguousarray(
                xT[b][:, hf * NQ:(hf + 1) * NQ]).astype(np_q),
            "xf": xT[b].astype(np_c),
            "qw": qw_t, "kw": kw_t, "vw": vw_t,
            "srw": srw_t, "srb": sr_b,
            "pw": pw_t, "pb": proj_b,
        })

    res = run_bass_kernel_spmd(nc, in_maps, core_ids=list(range(8)))
    _CACHE["last_exec_time_ns"] = res.exec_time_ns

    out = np.empty((B, N, C), dtype=np.float32)
    for c in range(8):
        b, hf = c // 2, c % 2
        out[b, hf * NQ:(hf + 1) * NQ, :] = res.results[c]["out_t"].T
    return out
